# revision 7
# baseline (speedup 1.0000x reference)
"""CCMLite kernel for Trainium2: GroupNorm(affine=False) + low-rank channel mix.

out = x_norm + u @ (v^T @ x_norm) + shift, with x_norm = groupnorm(x).

Sharding: data-parallel over batch B=16 across 8 cores (2 batch elems/core).

Pipeline-first redesign vs the previous version: the kernel is DMA-bound
(8.4 MB/core at ~360 GB/s ~= 23.5 us floor), so every phase is arranged to
keep the DMA engines streaming continuously:
  - all 8 x-tile loads issue up-front on the sync HWDGE ring
  - stats are split per-tile between DVE (bn_stats / 4x-mode sum-accum) and
    ACT (Square+accum_out), so per-batch stats lag the loads by ~2 us
  - stage A (vtx = vs^T x) packs 4 rank-12 strips per PSUM tile via
    tile_position; one wide DVE copy evacuates each column-half, and a tiny
    DMA restores the "ones" rows used to carry cst through stage B matmuls
  - stage B units of [128,1024] are routed per-unit to one of three paths:
      D: DVE scalar_tensor_tensor reads PSUM directly   (s*x + (u@vtx+cst))
      A: extra PE diag(s) matmul + ACT Identity+bias     (cheapest per-col)
      P: Pool (gpsimd) scalar_tensor_tensor              (otherwise-idle engine)
    so the combine work is spread across DVE/ACT/Pool under the DMA floor
  - each unit's [128,1024] output store DMAs immediately on the sync ring
"""

from contextlib import ExitStack

import numpy as np

import concourse.bass as bass
import concourse.tile as tile
from concourse import bacc, mybir
from concourse.bass_utils import run_bass_kernel_spmd

N_CORES = 8
B, C, H, W = 16, 256, 64, 64
HW = H * W            # 4096
R = 12                # low rank
G = 32                # groups
GPC = C // G          # 8 channels per group
P = 128               # partitions
CB = C // P           # 2 channel blocks
BPC = B // N_CORES    # 2 batch elements per core
EPS = 1e-6
F32 = mybir.dt.float32
F16 = mybir.dt.float16

_MULT = mybir.AluOpType.mult
_ADD = mybir.AluOpType.add
_SUB = mybir.AluOpType.subtract
AF = mybir.ActivationFunctionType

# ---- schedule knobs ----
# stats method per (b, cb): tuple over the two [128,2048] tiles,
#   'bn'  = DVE bn_stats (both moments, heavier on DVE)
#   'act' = ACT Square+accum for sumsq + DVE 4x-mode copy+accum for sum
DEF_STATS = {
    (0, 0): ("bn", "bn"),
    (0, 1): ("bn", "bn"),
    (1, 0): ("act", "act"),
    (1, 1): ("act", "act"),
}
# stage-B path per batch: 8 chars, unit order (k, cb) k-major.
#   D = DVE STT reads PSUM; A = PE diag + ACT Identity+bias;
#   E = DVE 4x t=s*x, ACT evac to SBUF fp16 (+cst bias), Pool TT-add
DEF_PATHS = ("DEDAEDAE", "DAEDAEAE")
DEF_WARM = 0  # PE warm-up matmuls before stage A


def build_nc(paths=DEF_PATHS, stats=DEF_STATS, warm=DEF_WARM):
    nc = bacc.Bacc(None, target_bir_lowering=False)
    x_d = nc.dram_tensor("x", [BPC, C, HW], F16, kind="ExternalInput")
    aug_d = nc.dram_tensor("aug", [BPC, P, C], F16, kind="ExternalInput")
    vsh_d = nc.dram_tensor("vsh", [BPC, CB, P, 33], F32, kind="ExternalInput")
    gmask_d = nc.dram_tensor("gmask", [P, 16], F32, kind="ExternalInput")
    gmaskT_d = nc.dram_tensor("gmaskT", [16, P], F32, kind="ExternalInput")
    ident_d = nc.dram_tensor("ident16", [P, P], F16, kind="ExternalInput")
    ones_d = nc.dram_tensor("ones16", [4, 512], F16, kind="ExternalInput")
    out_d = nc.dram_tensor("out", [BPC, C, HW], F16, kind="ExternalOutput")

    with tile.TileContext(nc) as tc, ExitStack() as ctx:
        consts = ctx.enter_context(tc.tile_pool(name="consts", bufs=1))
        xbp = ctx.enter_context(tc.tile_pool(name="xbp", bufs=8))
        junkp = ctx.enter_context(tc.tile_pool(name="junkp", bufs=2))
        outp = ctx.enter_context(tc.tile_pool(name="outp", bufs=6))
        vtp = ctx.enter_context(tc.tile_pool(name="vtp", bufs=3))
        smalls = ctx.enter_context(tc.tile_pool(name="smalls", bufs=2))
        ps_small = ctx.enter_context(
            tc.tile_pool(name="ps_small", bufs=2, space="PSUM"))
        ps_vtx = ctx.enter_context(tc.tile_pool(name="ps_vtx", bufs=1, space="PSUM"))
        ps_pm = ctx.enter_context(tc.tile_pool(name="ps_pm", bufs=2, space="PSUM"))

        # ---- consts ----
        gmask = consts.tile([P, 16], F32)
        nc.gpsimd.dma_start(out=gmask, in_=gmask_d[:, :])
        gmaskT = consts.tile([16, P], F32)
        nc.gpsimd.dma_start(out=gmaskT, in_=gmaskT_d[:, :])
        ident_h = consts.tile([P, P], F16)
        nc.gpsimd.dma_start(out=ident_h, in_=ident_d[:, :])
        ones4 = consts.tile([4, 512], F16)
        nc.gpsimd.dma_start(out=ones4, in_=ones_d[:, :])
        ones14 = consts.tile([1, 4], F16)
        nc.gpsimd.dma_start(out=ones14, in_=ones_d[0:1, 0:4])
        eps_t = consts.tile([16, 1], F32)
        nc.vector.memset(eps_t, EPS)

        # per-batch params on the scalar HWDGE ring (issued before x loads)
        augs, vshs = [], []
        for b in range(BPC):
            aug = smalls.tile([P, 2 * P], F16, tag=f"aug{b}", bufs=1)
            nc.scalar.dma_start(out=aug, in_=aug_d[b])
            vsh = smalls.tile([P, 66], F32, tag=f"vsh{b}", bufs=1)
            for cb in range(CB):
                nc.scalar.dma_start(
                    out=vsh[:, 33 * cb:33 * (cb + 1)], in_=vsh_d[b, cb])
            augs.append(aug)
            vshs.append(vsh)

        # warm the ACT tables early so Sqrt/Square don't pay a table load on
        # the critical path
        twarm = smalls.tile([16, 1], F32, tag="twarm", bufs=1)
        nc.scalar.activation(out=twarm, in_=eps_t, func=AF.Square)
        nc.scalar.activation(out=twarm, in_=eps_t, func=AF.Sqrt,
                             bias=eps_t[:, 0:1], scale=1.0)
        nc.scalar.activation(out=twarm, in_=eps_t, func=AF.Identity)

        # ---- all x loads up-front, sync ring ----
        xbt = {}
        for b in range(BPC):
            for cb in range(CB):
                for h in range(2):
                    tb = xbp.tile([P, 2048], F16, tag="xbt")
                    nc.sync.dma_start(
                        out=tb,
                        in_=x_d[b, cb * P:(cb + 1) * P, h * 2048:(h + 1) * 2048])
                    xbt[(b, cb, h)] = tb

        if warm:
            wps = ps_small.tile([P, 512], F32, tag="ps")
            for _ in range(warm):
                nc.tensor.matmul(wps[:, 0:P], lhsT=ident_h, rhs=ident_h,
                                 start=True, stop=True)

        # ---- per-(b,cb) stats + small chain ----
        sms = {}    # (b,cb) -> [128,2] f32: col0 rstd, col1 mean
        vss = {}    # (b,cb) -> [128,R] f16 (v*s)
        diags = {}  # (b,cb) -> [128,128] f16 diag(s)
        kvsb = {}   # (b,cb) -> [R,1] f32 partial kvec

        def emit_stats(b, cb):
            acc1 = smalls.tile([P, 2], F32, tag=f"acc1_{b}{cb}", bufs=1)
            acc2 = smalls.tile([P, 2], F32, tag=f"acc2_{b}{cb}", bufs=1)
            for t in range(2):
                xt = xbt[(b, cb, t)]
                if stats[(b, cb)][t] == "bn":
                    st = smalls.tile([P, 4, 6], F32, tag="st")
                    for i in range(4):
                        nc.vector.bn_stats(
                            out=st[:, i:i + 1, :], in_=xt[:, 512 * i:512 * (i + 1)])
                    mv = smalls.tile([P, 2], F32, tag="mv")
                    nc.vector.bn_aggr(out=mv, in_=st)
                    nc.vector.tensor_scalar_mul(
                        out=acc1[:, t:t + 1], in0=mv[:, 0:1], scalar1=2048.0)
                    # sumsq = (m^2 + v) * 2048
                    tmp = smalls.tile([P, 1], F32, tag="tmp")
                    nc.vector.scalar_tensor_tensor(
                        out=tmp, in0=mv[:, 0:1], scalar=mv[:, 0:1],
                        in1=mv[:, 1:2], op0=_MULT, op1=_ADD)
                    nc.vector.tensor_scalar_mul(
                        out=acc2[:, t:t + 1], in0=tmp, scalar1=2048.0)
                else:
                    ja = junkp.tile([P, 2048], F16, tag="ja")
                    nc.scalar.activation(
                        out=ja, in_=xt, func=AF.Square,
                        accum_out=acc2[:, t:t + 1])
                    jd = junkp.tile([P, 2048], F16, tag="jd")
                    nc.vector.tensor_scalar(
                        out=jd, in0=xt, scalar1=1.0, scalar2=0.0,
                        op0=_MULT, op1=_ADD, accum_out=acc1[:, t:t + 1])
            # per-channel mean / E[x^2]
            msum = smalls.tile([P, 2], F32, tag="msum")
            nc.vector.tensor_scalar(
                out=msum[:, 0:1], in0=acc1[:, 0:1], scalar1=acc1[:, 1:2],
                scalar2=1.0 / HW, op0=_ADD, op1=_MULT)
            nc.vector.tensor_scalar(
                out=msum[:, 1:2], in0=acc2[:, 0:1], scalar1=acc2[:, 1:2],
                scalar2=1.0 / HW, op0=_ADD, op1=_MULT)
            # group reduce + broadcast
            gs = ps_small.tile([16, 2], F32, tag="ps")
            nc.tensor.matmul(gs, lhsT=gmask, rhs=msum, start=True, stop=True)
            gvals = smalls.tile([16, 2], F32, tag="gvals")
            tmpg = smalls.tile([16, 4], F32, tag="tmpg")
            nc.vector.tensor_scalar_mul(
                out=gvals[:, 1:2], in0=gs[:, 0:1], scalar1=1.0 / GPC)
            nc.vector.tensor_scalar_mul(
                out=tmpg[:, 0:1], in0=gs[:, 1:2], scalar1=1.0 / GPC)
            nc.vector.tensor_mul(
                out=tmpg[:, 1:2], in0=gvals[:, 1:2], in1=gvals[:, 1:2])
            nc.vector.tensor_sub(
                out=tmpg[:, 2:3], in0=tmpg[:, 0:1], in1=tmpg[:, 1:2])
            nc.scalar.activation(
                out=tmpg[:, 3:4], in_=tmpg[:, 2:3], func=AF.Sqrt,
                bias=eps_t[:, 0:1], scale=1.0)
            nc.vector.reciprocal(out=gvals[:, 0:1], in_=tmpg[:, 3:4])
            bc = ps_small.tile([P, 2], F32, tag="ps")
            nc.tensor.matmul(bc, lhsT=gmaskT, rhs=gvals, start=True, stop=True)
            sm = smalls.tile([P, 2], F32, tag=f"sm{b}{cb}", bufs=1)
            nc.vector.tensor_copy(out=sm, in_=bc)
            sms[(b, cb)] = sm
            # vs = v * s, zero-padded to 32 cols so stage A strips write all
            # 32 partitions (avoids reading uninitialized PSUM in the evac)
            vsf = smalls.tile([P, 32], F32, tag=f"vsf{b}{cb}", bufs=1)
            nc.vector.tensor_scalar_mul(
                out=vsf, in0=vshs[b][:, 33 * cb:33 * cb + 32],
                scalar1=sm[:, 0:1])
            vs = smalls.tile([P, 32], F16, tag=f"vs{b}{cb}", bufs=1)
            nc.vector.tensor_copy(out=vs, in_=vsf)
            vss[(b, cb)] = vs
            if "A" in paths[b]:
                diag = smalls.tile([P, P], F16, tag=f"diag{b}{cb}", bufs=1)
                nc.vector.tensor_scalar_mul(out=diag, in0=ident_h,
                                            scalar1=sm[:, 0:1])
                diags[(b, cb)] = diag
            # kvec partial: kv[r] = sum_c vs[c,r]*m_c
            kv = ps_small.tile([32, 1], F32, tag="ps")
            nc.tensor.matmul(kv, lhsT=vsf, rhs=sm[:, 1:2], start=True, stop=True)
            kvp = smalls.tile([R, 1], F32, tag=f"kv{b}{cb}", bufs=1)
            nc.vector.tensor_copy(out=kvp, in_=kv[0:R, :])
            kvsb[(b, cb)] = kvp

        def emit_cst(b):
            # cst_c = shift_c - m_c*s_c - sum_r u[c,r]*kvec[r]; write into
            # aug row R via transpose + SBUF->SBUF DMA
            kvs = smalls.tile([R, 1], F32, tag=f"kvs{b}", bufs=1)
            nc.vector.tensor_add(out=kvs, in0=kvsb[(b, 0)], in1=kvsb[(b, 1)])
            kvs16 = smalls.tile([R, 1], F16, tag=f"kvs16{b}", bufs=1)
            nc.vector.tensor_copy(out=kvs16, in_=kvs)
            for cb in range(CB):
                aug = augs[b]
                sm = sms[(b, cb)]
                ukv = ps_small.tile([P, 1], F32, tag="ps")
                nc.tensor.matmul(
                    ukv, lhsT=aug[0:R, P * cb:P * (cb + 1)], rhs=kvs16,
                    start=True, stop=True)
                cst = smalls.tile([P, 1], F32, tag=f"cst{b}{cb}", bufs=1)
                nc.vector.tensor_mul(out=cst, in0=sm[:, 1:2], in1=sm[:, 0:1])
                nc.vector.tensor_sub(
                    out=cst,
                    in0=vshs[b][:, 33 * cb + 32:33 * cb + 33],
                    in1=cst)
                nc.vector.tensor_sub(out=cst, in0=cst, in1=ukv)
                csts[(b, cb)] = cst
                cst16 = smalls.tile([P, 1], F16, tag="cst16")
                nc.vector.tensor_copy(out=cst16, in_=cst)
                ctp = ps_small.tile([1, P], F16, tag="ps")
                nc.tensor.transpose(out=ctp, in_=cst16, identity=ident_h)
                cstrow = smalls.tile([1, P], F16, tag="cstrow")
                nc.scalar.copy(out=cstrow, in_=ctp)
                ctp4 = ps_small.tile([4, P], F32, tag="ps")
                nc.tensor.matmul(ctp4, lhsT=ones14, rhs=cstrow,
                                 start=True, stop=True)
                cstrow4 = smalls.tile([4, P], F16, tag="cstrow4")
                nc.scalar.copy(out=cstrow4, in_=ctp4)
                pstride = aug.ap[0][0]
                dst = bass.AP(
                    tensor=aug.tensor,
                    offset=aug.offset + R * pstride + P * cb,
                    ap=[[32 * pstride, 4], [1, P]])
                nc.gpsimd.dma_start(out=dst, in_=cstrow4)

        csts = {}

        def emit_stage_a(b, ch):
            # vtx strips for chunks j = 4*ch + q into ps tile [128,512];
            # strip q occupies partitions 32q..32q+12
            vps = vtx_ps[b]
            for cb in range(CB):
                for q in range(4):
                    nc.tensor.matmul(
                        vps[32 * q:32 * q + 32, 512 * ch:512 * (ch + 1)],
                        lhsT=vss[(b, cb)],
                        rhs=xbt[(b, cb, ch)][:, 512 * q:512 * (q + 1)],
                        start=(cb == 0), stop=(cb == CB - 1),
                        tile_position=(0, 32 * q),
                        skip_group_check=True)

        def emit_evac(b, ch):
            vt = vtp.tile([P, 512], F16, tag="vt")
            nc.vector.tensor_copy(
                out=vt, in_=vtx_ps[b][:, 512 * ch:512 * (ch + 1)])
            # restore ones rows (partitions 12,44,76,108) clobbered by the
            # full-tile copy; engines can't write at partition offset 12, DMA can
            pstride = vt.ap[0][0]
            dst = bass.AP(
                tensor=vt.tensor,
                offset=vt.offset + 12 * pstride,
                ap=[[32 * pstride, 4], [1, 512]])
            nc.gpsimd.dma_start(out=dst, in_=ones4[:, :])
            vts[(b, ch)] = vt

        vts = {}
        vtx_ps = {}

        def emit_unit(b, k, cb):
            # output unit [128,1024]: chunks (2k, 2k+1); x tile h=k//2,
            # cols (k%2)*1024; vtx strips q = 2k%4, (2k+1)%4 in vts[(b, k//2... )]
            path = paths[b][2 * k + cb]
            h, half = k // 2, k % 2
            aug = augs[b]
            sm = sms[(b, cb)]
            x_ap = xbt[(b, cb, h)][:, 1024 * half:1024 * (half + 1)]
            pm = ps_pm.tile([P, 1024], F32, tag="pm")
            for j2 in range(2):
                q = 2 * half + j2
                vt = vts[(b, h)]
                pslice = pm[:, 512 * j2:512 * (j2 + 1)]
                if path == "A":
                    nc.tensor.matmul(
                        pslice, lhsT=diags[(b, cb)],
                        rhs=xbt[(b, cb, h)][:, 512 * q:512 * (q + 1)],
                        start=True, stop=False,
                        skip_group_check=True)
                    nc.tensor.matmul(
                        pslice,
                        lhsT=aug[32 * q:32 * q + R, P * cb:P * (cb + 1)],
                        rhs=vt[32 * q:32 * q + R, :],
                        start=False, stop=True,
                        tile_position=(32 * q, 0),
                        skip_group_check=True)
                else:
                    nc.tensor.matmul(
                        pslice,
                        lhsT=aug[32 * q:32 * q + R + 1, P * cb:P * (cb + 1)],
                        rhs=vt[32 * q:32 * q + R + 1, :],
                        start=True, stop=True,
                        tile_position=(32 * q, 0),
                        skip_group_check=True)
            osb = outp.tile([P, 1024], F16, tag="osb")
            if path == "A":
                nc.scalar.activation(
                    out=osb, in_=pm, func=AF.Identity,
                    bias=csts[(b, cb)], scale=1.0)
            elif path == "D":
                nc.vector.scalar_tensor_tensor(
                    out=osb, in0=x_ap, scalar=sm[:, 0:1], in1=pm,
                    op0=_MULT, op1=_ADD)
            else:  # E: DVE 4x t=s*x, ACT evac (+cst bias), Pool TT-add
                t = outp.tile([P, 1024], F16, tag="tsx", bufs=3)
                nc.vector.tensor_scalar(
                    out=t, in0=x_ap, scalar1=sm[:, 0:1], scalar2=0.0,
                    op0=_MULT, op1=_ADD)
                pmsb = outp.tile([P, 1024], F16, tag="pmsb", bufs=3)
                nc.scalar.activation(
                    out=pmsb, in_=pm, func=AF.Identity,
                    bias=csts[(b, cb)], scale=1.0)
                nc.gpsimd.tensor_add(out=osb, in0=t, in1=pmsb)
            nc.sync.dma_start(
                out=out_d[b, cb * P:(cb + 1) * P, 1024 * k:1024 * (k + 1)],
                in_=osb)

        # ================= schedule =================
        # batch 0 head
        vtx_ps[0] = ps_vtx.tile([P, 1024], F32, tag="vtx", name="vtx0")
        emit_stats(0, 0)
        emit_stats(0, 1)
        emit_cst(0)
        emit_stage_a(0, 0)
        emit_evac(0, 0)
        emit_stage_a(0, 1)
        # b1 stats for cb0 land during b0 compute; emit before b0 evac/combines
        emit_stats(1, 0)
        emit_evac(0, 1)
        emit_stats(1, 1)
        emit_cst(1)
        # b0 stage B
        for k in range(4):
            for cb in range(CB):
                emit_unit(0, k, cb)
        # b1 tail
        vtx_ps[1] = ps_vtx.tile([P, 1024], F32, tag="vtx", name="vtx1")
        emit_stage_a(1, 0)
        emit_evac(1, 0)
        emit_stage_a(1, 1)
        emit_evac(1, 1)
        for k in range(4):
            for cb in range(CB):
                emit_unit(1, k, cb)

    nc.finalize()
    return nc


def _host_prep(x, ccm_params):
    x = np.asarray(x, dtype=np.float32).reshape(B, C, HW).astype(np.float16)
    x = np.ascontiguousarray(x)
    cp = np.asarray(ccm_params, dtype=np.float32)
    u = cp[:, :C * R].reshape(B, C, R)
    v = cp[:, C * R:2 * C * R].reshape(B, C, R)
    shift = cp[:, 2 * C * R:].reshape(B, C)
    # aug: [B, 128, C] fp16; strips s=0..3: rows 32s..32s+11 = u^T,
    # row 32s+12 = cst written on device
    aug = np.zeros((B, P, C), np.float16)
    ut = u.transpose(0, 2, 1).astype(np.float16)
    for sx in range(4):
        aug[:, 32 * sx:32 * sx + R, :] = ut
    aug = np.ascontiguousarray(aug)
    # vsh: [B, CB, P, 33] f32: cols 0..11 = v, 12..31 zero pad, col 32 = shift
    vsh = np.zeros((B, CB, P, 33), np.float32)
    vsh[..., :R] = v.reshape(B, CB, P, R)
    vsh[..., 32] = shift.reshape(B, CB, P)
    vsh = np.ascontiguousarray(vsh)
    gmask = np.zeros((P, 16), np.float32)
    gmask[np.arange(P), np.arange(P) // GPC] = 1.0
    gmaskT = np.ascontiguousarray(gmask.T)
    ident16 = np.eye(P, dtype=np.float16)
    ones16 = np.ones((4, 512), np.float16)
    in_maps = []
    for c in range(N_CORES):
        bs = slice(c * BPC, (c + 1) * BPC)
        in_maps.append({
            "x": x[bs], "aug": aug[bs], "vsh": vsh[bs],
            "gmask": gmask, "gmaskT": gmaskT, "ident16": ident16,
            "ones16": ones16,
        })
    return in_maps


def kernel(x, ccm_params, _trace=False, _paths=DEF_PATHS, _stats=DEF_STATS,
           _warm=DEF_WARM, **_ignored):
    in_maps = _host_prep(x, ccm_params)
    nc = build_nc(paths=_paths, stats=_stats, warm=_warm)
    res = run_bass_kernel_spmd(
        nc, in_maps, core_ids=list(range(N_CORES)), trace=_trace)
    out = np.concatenate([r["out"] for r in res.results], axis=0)
    out = out.reshape(B, C, H, W).astype(np.float32, copy=False)
    if _trace:
        return out, res
    return out
